# revision 22
# baseline (speedup 1.0000x reference)
"""Block-tridiagonal iterative MLP on 8 TRN2 NeuronCores.

Tensor-parallel split of every W block along the output-feature dim (256
features per core). All matmuls run in fp8 with DoubleRow perf mode (2
contraction tiles per instruction at 0.5 cycles/row = 2x bf16 FLOP rate).
Accuracy is recovered with a residual split: every operand T is stored as
hi = e4m3(T*s) plus lo = e5m2(T*s - hi) at the SAME scale (e5m2's wider
exponent absorbs the 2^-4 magnitude drop), so the three cross terms
hi@Whi + hi@Wlo + lo@Whi accumulate in a single PSUM group with no fixup.
Iteration 2 drops the lo-correction on 2 of 16 contraction tiles (DROP2),
trading a measured rel err of 0.33% -> 1.40% (gate 2%) for ~4us.

Schedule: iteration 1 runs row-major (each block row finishes early so its
hi/lo activations can be AllGathered per block while later rows compute);
iteration 2 orders block-pairs by gathered-input availability (all pairs
contracting block 0 first, then block 1, ...), which lets the Tensor
engine run the inter-iteration boundary without a single idle cycle.
The relu, bias add, and fp8 re-quantization of iteration-1 outputs run on
the Scalar and Vector engines, fully overlapped with the Tensor engine.
"""
import sys

sys.path.insert(0, "/opt/trn_rl_repo")

import numpy as np
import ml_dtypes

import concourse.bass as bass
import concourse.mybir as mybir
from concourse.bass_utils import run_bass_kernel_spmd

N_CORES = 8
NUM_BLOCKS = 4
BLOCK_SIZE = 2048
BATCH = 512
BLOCK_PAIRS = [(0, 0), (0, 1), (1, 0), (1, 1), (1, 2),
               (2, 1), (2, 2), (2, 3), (3, 2), (3, 3)]
ROWS = {i: [(k, j) for k, (ii, j) in enumerate(BLOCK_PAIRS) if ii == i]
        for i in range(NUM_BLOCKS)}
ROW_OF = {k: i for k, (i, _) in enumerate(BLOCK_PAIRS)}
J_OF = {k: j for k, (_, j) in enumerate(BLOCK_PAIRS)}

P = 128
OSL = BLOCK_SIZE // N_CORES          # 256 out features per core
NOT = OSL // P                       # 2 output tiles (PSUM groups) per block
NET = BLOCK_SIZE // P                # 16 contraction tiles
NPAIR = NET // 2                     # 8 DoubleRow instructions per pass
SX = 16.0                            # activation scale into fp8 units
SW = 32.0                            # weight scale into fp8 units
DROP2 = 2                            # k-tiles (of 16) dropped from iter-2's
                                     # lo-correction passes; rel err 0.33%->1.40%
                                     # (measured on hw), 30% under the 2e-2 gate
E4 = mybir.dt.float8e4
E5 = mybir.dt.float8e5
BF = mybir.dt.bfloat16
F32 = mybir.dt.float32
E4np = ml_dtypes.float8_e4m3
E5np = ml_dtypes.float8_e5m2

# iteration-1 chunk schedule: row-major-ish, passes A (hi@Whi), B (hi@Wlo),
# C (lo@Whi); a chunk is (pass, pair) and emits 16 DoubleRow matmuls.  The
# order is matched to DMA arrival: C chunks of pairs contracting an
# already-resident block serve as fillers while the next loads land.
ITER1_CHUNK_ORDER = [
    ("A", 0), ("A", 2), ("A", 1), ("B", 0), ("B", 1), ("C", 0), ("C", 1),
    ("C", 2), ("B", 2), ("A", 3), ("B", 3), ("C", 3), ("A", 4), ("B", 4),
    ("C", 4),
    ("A", 5), ("C", 5), ("B", 5), ("A", 6), ("B", 6), ("C", 6), ("A", 7),
    ("B", 7), ("C", 7),
    ("A", 8), ("A", 9), ("B", 8), ("B", 9), ("C", 8), ("C", 9),
]
ITER1_CHUNKS = [(p, k, J_OF[k]) for p, k in ITER1_CHUNK_ORDER]
ITER1_LOAD_ORDER = [
    ("ah", 0), ("wh", 0), ("wh", 2), ("ah", 1), ("wh", 1),
    ("wl", 0), ("wl", 1), ("al", 0), ("al", 1),
    ("wl", 2), ("wh", 3), ("wl", 3), ("ah", 2), ("wh", 4), ("wl", 4),
    ("al", 2),
    ("wh", 5), ("wl", 5), ("wh", 6), ("wl", 6), ("ah", 3), ("wh", 7),
    ("wl", 7), ("al", 3),
    ("wh", 8), ("wh", 9), ("wl", 8), ("wl", 9),
]
# iteration-2: pairs grouped by which gathered block they contract, so the
# PE never waits on the AllGather chain.  Within the last group k9 (row 3)
# precedes k7 (row 2) being swapped would break bank-stop order; keep k7
# after k4/k6's group and k9 last so banks stop in ascending order.
ITER2_PAIR_PHASES = [[0, 2], [1, 3, 5], [4, 6, 8], [7, 9]]
ITER2_CHUNKS = [(p, k, J_OF[k])
                for phase in ITER2_PAIR_PHASES
                for k in phase
                for p in "ABC"]

# last iter-1 row whose matmuls read activation block j (for a1 WAR)
LAST_ROW_READING = {j: min(j + 1, NUM_BLOCKS - 1) for j in range(NUM_BLOCKS)}


def _bank_schedule(chunks):
    """first/last (pass,pair,ot) touches per PSUM bank + stop order."""
    first, last = {}, {}
    for ci, (p, k, j) in enumerate(chunks):
        for ot in range(NOT):
            g = 2 * ROW_OF[k] + ot
            first.setdefault(g, (ci, ot))
            last[g] = (ci, ot)
    stop_order = sorted(last, key=lambda g: (last[g][0], last[g][1]))
    return first, last, stop_order


I1_FIRST, I1_LAST, I1_STOPS = _bank_schedule(ITER1_CHUNKS)
I2_FIRST, I2_LAST, I2_STOPS = _bank_schedule(ITER2_CHUNKS)
assert I1_STOPS == list(range(8)), I1_STOPS
# pe_sem value after bank g's stop, per iteration
I1_STOPV = {g: I1_STOPS.index(g) + 1 for g in range(8)}
I2_STOPV = {g: 8 + I2_STOPS.index(g) + 1 for g in range(8)}


def build_nc(mock_cc=False):
    nc = bass.Bass(num_devices=N_CORES)

    wh = nc.dram_tensor("wh", [10, P, NET, OSL], E4, kind="ExternalInput")
    wl = nc.dram_tensor("wl", [10, P, NET, OSL], E5, kind="ExternalInput")
    ah = nc.dram_tensor("ah", [NUM_BLOCKS, P, NET, BATCH], E4, kind="ExternalInput")
    al = nc.dram_tensor("al", [NUM_BLOCKS, P, NET, BATCH], E5, kind="ExternalInput")
    b1 = nc.dram_tensor("b1", [P, 2 * NUM_BLOCKS], F32, kind="ExternalInput")
    b2 = nc.dram_tensor("b2", [P, 2 * NUM_BLOCKS], F32, kind="ExternalInput")
    y_out = nc.dram_tensor("y", [NUM_BLOCKS, NOT, P, BATCH], BF, kind="ExternalOutput")

    cc_inh = nc.dram_tensor("cc_inh", [NUM_BLOCKS, NOT, P, BATCH], E4)
    cc_inl = nc.dram_tensor("cc_inl", [NUM_BLOCKS, NOT, P, BATCH], E5)
    cc_outh = nc.dram_tensor("cc_outh", [NUM_BLOCKS, BLOCK_SIZE, BATCH], E4,
                             addr_space="Shared")
    cc_outl = nc.dram_tensor("cc_outl", [NUM_BLOCKS, BLOCK_SIZE, BATCH], E5,
                             addr_space="Shared")

    CC_BLK = 32 if mock_cc else 2    # cc_sem per gathered block

    with (
        nc.sbuf_tensor("wh_sb", [P, 10, NET, OSL], E4) as wh_sb,
        nc.sbuf_tensor("wl_sb", [P, 10, NET, OSL], E5) as wl_sb,
        nc.sbuf_tensor("ah_sb", [P, NUM_BLOCKS, NET, BATCH], E4) as ah_sb,
        nc.sbuf_tensor("al_sb", [P, NUM_BLOCKS, NET, BATCH], E5) as al_sb,
        nc.sbuf_tensor("b1_sb", [P, 2 * NUM_BLOCKS], F32) as b1_sb,
        nc.sbuf_tensor("b2_sb", [P, 2 * NUM_BLOCKS], F32) as b2_sb,
        nc.sbuf_tensor("hi_sb", [P, 2 * NUM_BLOCKS, BATCH], E4) as hi_sb,
        nc.sbuf_tensor("lo_sb", [P, 2 * NUM_BLOCKS, BATCH], E5) as lo_sb,
        nc.sbuf_tensor("rf_sb", [P, 2, BATCH], BF) as rf_sb,
        nc.sbuf_tensor("y_sb", [P, 4, BATCH], BF) as y_sb,
        nc.psum_tensor("ps", [P, 2 * NUM_BLOCKS, BATCH], F32) as ps,
        nc.Block() as block,
    ):
        import contextlib
        _st = contextlib.ExitStack()
        wsem = [_st.enter_context(nc.semaphore(f"wsem{k}")) for k in range(10)]
        asem = [_st.enter_context(nc.semaphore(f"asem{j}")) for j in range(4)]
        bias_sem = _st.enter_context(nc.semaphore("bias_sem"))
        pe_sem = _st.enter_context(nc.semaphore("pe_sem"))
        act1_sem = _st.enter_context(nc.semaphore("act1_sem"))
        act2_sem = _st.enter_context(nc.semaphore("act2_sem"))
        dve_sem = _st.enter_context(nc.semaphore("dve_sem"))
        cin_sem = _st.enter_context(nc.semaphore("cin_sem"))
        cc_sem = _st.enter_context(nc.semaphore("cc_sem"))
        out_sem = _st.enter_context(nc.semaphore("out_sem"))

        @block.sync
        def _(sp: bass.BassEngine):
            def ld_w(k, hi):
                src, dst = (wh, wh_sb) if hi else (wl, wl_sb)
                sp.dma_start(dst[:, k, :, :], src[k]).then_inc(wsem[k], 16)

            def ld_a(j, hi):
                src, dst = (ah, ah_sb) if hi else (al, al_sb)
                sp.dma_start(dst[:, j, :, :], src[j]).then_inc(asem[j], 16)

            # iteration-1 loads, ordered to keep the PE fed from its delayed
            # start onward (wsem: 16=hi 32=lo; asem: 16=hi 32=lo)
            for n, (kind, idx) in enumerate(ITER1_LOAD_ORDER):
                if kind in ("wh", "wl"):
                    ld_w(idx, kind == "wh")
                else:
                    ld_a(idx, kind == "ah")
                if n == 2:
                    sp.dma_start(b1_sb[:, :], b1[:, :]).then_inc(bias_sem, 16)
                    sp.dma_start(b2_sb[:, :], b2[:, :]).then_inc(bias_sem, 16)

            # iteration-1 activations out to the collective, per block
            def cc_write(i):
                sp.wait_ge(dve_sem, 2 * (i + 1))
                for ot in range(NOT):
                    sp.dma_start(cc_inh[i, ot], hi_sb[:, 2 * i + ot, :]
                                 ).then_inc(cin_sem, 16)
                for ot in range(NOT):
                    sp.dma_start(cc_inl[i, ot], lo_sb[:, 2 * i + ot, :]
                                 ).then_inc(cin_sem, 16)

            # gathered re-loads (asem: 48=hi 64=lo); overwrites iter-1 slot j
            def a1_load(j):
                if not mock_cc:
                    sp.wait_ge(cc_sem, CC_BLK * (j + 1))
                sp.wait_ge(pe_sem, 2 * (LAST_ROW_READING[j] + 1))
                sp.dma_start(
                    ah_sb[:, j, :, :],
                    cc_outh[j].rearrange("(et p) b -> p et b", p=P),
                ).then_inc(asem[j], 16)
                sp.dma_start(
                    al_sb[:, j, :, :],
                    cc_outl[j].rearrange("(et p) b -> p et b", p=P),
                ).then_inc(asem[j], 16)

            for i in range(NUM_BLOCKS):
                cc_write(i)
                if mock_cc:
                    # collective stand-in on the sp queue: a local DMA of the
                    # same byte volume.  DMA-queue FIFO order makes the
                    # cc_in -> mock -> a1 data chain safe without semaphores.
                    sp.dma_start(
                        cc_outh[i, 0:NOT * P],
                        cc_inh[i].rearrange("t p b -> (t p) b"),
                    ).then_inc(cc_sem, 16)
                    sp.dma_start(
                        cc_outl[i, 0:NOT * P],
                        cc_inl[i].rearrange("t p b -> (t p) b"),
                    ).then_inc(cc_sem, 16)
                a1_load(i)

            for n, g in enumerate(I2_STOPS):
                i, ot = g // 2, g % 2
                sp.wait_ge(act2_sem, n + 1)
                sp.dma_start(y_out[i, ot], y_sb[:, n % 4, :]).then_inc(out_sem, 16)

        if not mock_cc:
            @block.gpsimd
            def _(gp: bass.BassGpSimd):
                for i in range(NUM_BLOCKS):
                    gp.wait_ge(cin_sem, 64 * (i + 1))
                    gp.collective_compute(
                        "AllGather",
                        mybir.AluOpType.bypass,
                        replica_groups=[list(range(N_CORES))],
                        ins=[cc_inh[i].opt()],
                        outs=[cc_outh[i].opt()],
                    ).then_inc(cc_sem, 1)
                    gp.collective_compute(
                        "AllGather",
                        mybir.AluOpType.bypass,
                        replica_groups=[list(range(N_CORES))],
                        ins=[cc_inl[i].opt()],
                        outs=[cc_outl[i].opt()],
                    ).then_inc(cc_sem, 1)

        @block.tensor
        def _(pe: bass.BassTensorEngine):
            DR = mybir.MatmulPerfMode.DoubleRow
            started = set()
            for it, chunks, first, last, stopv in (
                    (0, ITER1_CHUNKS, I1_FIRST, I1_LAST, I1_STOPV),
                    (1, ITER2_CHUNKS, I2_FIRST, I2_LAST, I2_STOPV)):
                for ci, (pss, k, j) in enumerate(chunks):
                    for ot in range(NOT):
                        g = 2 * ROW_OF[k] + ot
                        # operand arrival waits double as enqueue pacing
                        if it == 0:
                            if pss == "A":
                                pe.wait_ge(wsem[k], 16)
                                pe.wait_ge(asem[j], 16)
                            elif pss == "B":
                                pe.wait_ge(wsem[k], 32)
                            else:
                                pe.wait_ge(asem[j], 32)
                        else:
                            pe.wait_ge(asem[j], 48 if pss in "AB" else 64)
                        if it == 1 and (it, g) not in started:
                            pe.wait_ge(act1_sem, 2 * g + 2)  # PSUM WAR vs act1
                        xs = ah_sb if pss in "AB" else al_sb
                        ws = wh_sb if pss in "AC" else wl_sb
                        is_last = last[g] == (ci, ot)
                        n_instr = NPAIR
                        if it == 1 and pss in "BC":
                            n_instr = NPAIR - DROP2 // 2
                        for ei in range(n_instr):
                            et = 2 * ei
                            mm = pe.matmul(
                                ps[:, g, :],
                                ws[:, k, et:et + 2, ot * P:(ot + 1) * P],
                                xs[:, j, et:et + 2, :],
                                start=(it, g) not in started,
                                stop=is_last and ei == n_instr - 1,
                                perf_mode=DR,
                                skip_group_check=True,
                            )
                            started.add((it, g))
                            if is_last and ei == n_instr - 1:
                                mm.then_inc(pe_sem, 1)

        @block.scalar
        def _(ac: bass.BassScalarEngine):
            Relu = mybir.ActivationFunctionType.Relu
            ac.wait_ge(bias_sem, 32)
            for g in range(2 * NUM_BLOCKS):
                ac.wait_ge(pe_sem, I1_STOPV[g])
                ac.activation(hi_sb[:, g, :], ps[:, g, :], Relu,
                              bias=b1_sb[:, g:g + 1],
                              scale=1.0 / SW).then_inc(act1_sem, 1)
                if g >= 2:
                    ac.wait_ge(dve_sem, g - 1)  # rf buf WAR vs DVE of g-2
                ac.activation(rf_sb[:, g % 2, :], ps[:, g, :], Relu,
                              bias=b1_sb[:, g:g + 1],
                              scale=1.0 / SW).then_inc(act1_sem, 1)
            for n, g in enumerate(I2_STOPS):
                ac.wait_ge(pe_sem, I2_STOPV[g])
                if n >= 4:
                    ac.wait_ge(out_sem, 16 * (n - 3))  # y buf WAR vs store
                ac.activation(y_sb[:, n % 4, :], ps[:, g, :], Relu,
                              bias=b2_sb[:, g:g + 1],
                              scale=1.0 / (SX * SW)).then_inc(act2_sem, 1)

        @block.vector
        def _(ve):
            for g in range(2 * NUM_BLOCKS):
                ve.wait_ge(act1_sem, 2 * g + 2)
                ve.tensor_sub(lo_sb[:, g, :], rf_sb[:, g % 2, :],
                              hi_sb[:, g, :]).then_inc(dve_sem, 1)

    return nc


def _prep_inputs(X, W, b):
    """Host-side shard/layout prep (pure numpy, per-core views)."""
    Xs = np.ascontiguousarray(
        (X.astype(np.float32) * SX)
        .reshape(NUM_BLOCKS, BATCH, NET, P).transpose(0, 3, 2, 1))
    ah = Xs.astype(E4np)
    al = (Xs - ah.astype(np.float32)).astype(E5np)

    Ws = W.astype(np.float32) * SW
    Whi = Ws.astype(E4np)
    Wlo = (Ws - Whi.astype(np.float32)).astype(E5np)

    B = np.zeros((NUM_BLOCKS, BLOCK_SIZE), dtype=np.float32)
    for k, (i, _) in enumerate(BLOCK_PAIRS):
        B[i] += b[k]

    in_maps = []
    for c in range(N_CORES):
        sl = slice(c * OSL, (c + 1) * OSL)
        whc = np.ascontiguousarray(
            Whi[:, sl, :].reshape(10, OSL, NET, P).transpose(0, 3, 2, 1))
        wlc = np.ascontiguousarray(
            Wlo[:, sl, :].reshape(10, OSL, NET, P).transpose(0, 3, 2, 1))
        # bias per group column g = 2*i + ot: value for partition p is
        # B[i, c*256 + ot*128 + p]
        bcols = B[:, sl].reshape(2 * NUM_BLOCKS, P).T   # [p, g]
        in_maps.append({
            "wh": whc, "wl": wlc, "ah": ah, "al": al,
            "b1": np.ascontiguousarray(bcols * SX, dtype=np.float32),
            "b2": np.ascontiguousarray(bcols, dtype=np.float32),
        })
    return in_maps


_CACHE = {}


def kernel(X, W, b):
    X = np.asarray(X, dtype=np.float32)
    W = np.asarray(W, dtype=np.float32)
    b = np.asarray(b, dtype=np.float32)
    in_maps = _prep_inputs(X, W, b)
    if "nc" not in _CACHE:
        _CACHE["nc"] = build_nc()
    res = run_bass_kernel_spmd(_CACHE["nc"], in_maps,
                               core_ids=list(range(N_CORES)))
    out = np.empty((NUM_BLOCKS, BATCH, BLOCK_SIZE), dtype=np.float32)
    for c in range(N_CORES):
        y = res.results[c]["y"].astype(np.float32)    # [4, 2, 128, 512] bf16
        out[:, :, c * OSL:(c + 1) * OSL] = y.transpose(0, 3, 1, 2).reshape(
            NUM_BLOCKS, BATCH, OSL)
    return out


# revision 27
# speedup vs baseline: 1.0068x; 1.0068x over previous
"""Block-tridiagonal iterative MLP on 8 TRN2 NeuronCores.

Tensor-parallel split of every W block along the output-feature dim (256
features per core). All matmuls run in fp8 with DoubleRow perf mode (2
contraction tiles per instruction at 0.5 cycles/row = 2x bf16 FLOP rate).
Accuracy is recovered with a residual split: every operand T is stored as
hi = e4m3(T*s) plus lo = e5m2(T*s - hi) at the SAME scale (e5m2's wider
exponent absorbs the 2^-4 magnitude drop), so the three cross terms
hi@Whi + hi@Wlo + lo@Whi accumulate in a single PSUM group with no fixup.
Iteration 2 drops the lo-correction on 2 of 16 contraction tiles (DROP2),
trading a measured rel err of 0.33% -> 1.40% (gate 2%) for ~4us.

Schedule: iteration 1 runs row-major (each block row finishes early so its
hi/lo activations can be AllGathered per block while later rows compute);
iteration 2 orders block-pairs by gathered-input availability (all pairs
contracting block 0 first, then block 1, ...), which lets the Tensor
engine run the inter-iteration boundary without a single idle cycle.
The relu, bias add, and fp8 re-quantization of iteration-1 outputs run on
the Scalar and Vector engines, fully overlapped with the Tensor engine.
"""
import sys

sys.path.insert(0, "/opt/trn_rl_repo")

import numpy as np
import ml_dtypes

import concourse.bass as bass
import concourse.mybir as mybir
from concourse.bass_utils import run_bass_kernel_spmd

N_CORES = 8
NUM_BLOCKS = 4
BLOCK_SIZE = 2048
BATCH = 512
BLOCK_PAIRS = [(0, 0), (0, 1), (1, 0), (1, 1), (1, 2),
               (2, 1), (2, 2), (2, 3), (3, 2), (3, 3)]
ROWS = {i: [(k, j) for k, (ii, j) in enumerate(BLOCK_PAIRS) if ii == i]
        for i in range(NUM_BLOCKS)}
ROW_OF = {k: i for k, (i, _) in enumerate(BLOCK_PAIRS)}
J_OF = {k: j for k, (_, j) in enumerate(BLOCK_PAIRS)}

P = 128
OSL = BLOCK_SIZE // N_CORES          # 256 out features per core
NOT = OSL // P                       # 2 output tiles (PSUM groups) per block
NET = BLOCK_SIZE // P                # 16 contraction tiles
NPAIR = NET // 2                     # 8 DoubleRow instructions per pass
SX = 16.0                            # activation scale into fp8 units
SW = 32.0                            # weight scale into fp8 units
DROP2 = 2                            # k-tiles (of 16) dropped from iter-2's
                                     # lo-correction passes; rel err 0.33%->1.40%
                                     # (measured on hw), 30% under the 2e-2 gate
E4 = mybir.dt.float8e4
E5 = mybir.dt.float8e5
BF = mybir.dt.bfloat16
F32 = mybir.dt.float32
E4np = ml_dtypes.float8_e4m3
E5np = ml_dtypes.float8_e5m2

# iteration-1 chunk schedule: row-major-ish, passes A (hi@Whi), B (hi@Wlo),
# C (lo@Whi); a chunk is (pass, pair) and emits 16 DoubleRow matmuls.  The
# order is matched to DMA arrival: C chunks of pairs contracting an
# already-resident block serve as fillers while the next loads land.
ITER1_CHUNK_ORDER = [
    ("A", 0), ("A", 2), ("A", 1), ("B", 0), ("B", 1), ("B", 2), ("A", 3),
    ("C", 0), ("B", 3), ("C", 1), ("C", 2), ("C", 3), ("A", 4), ("B", 4),
    ("A", 5), ("C", 4), ("B", 5), ("C", 5), ("A", 6), ("B", 6), ("C", 6),
    ("A", 7), ("B", 7), ("C", 7),
    ("A", 8), ("A", 9), ("B", 8), ("B", 9), ("C", 8), ("C", 9),
]
ITER1_CHUNKS = [(p, k, J_OF[k]) for p, k in ITER1_CHUNK_ORDER]
ITER1_LOAD_ORDER = [
    ("ah", 0), ("wh", 0), ("wh", 2), ("ah", 1), ("wh", 1),
    ("wl", 0), ("wl", 1), ("wl", 2), ("wh", 3), ("al", 0), ("wl", 3),
    ("al", 1), ("ah", 2), ("wh", 4), ("wl", 4), ("wh", 5), ("al", 2),
    ("wl", 5), ("wh", 6), ("wl", 6), ("ah", 3), ("wh", 7), ("wl", 7),
    ("al", 3), ("wh", 8), ("wh", 9), ("wl", 8), ("wl", 9),
]
# iteration-2: pairs grouped by which gathered block they contract, so the
# PE never waits on the AllGather chain.  Within the last group k9 (row 3)
# precedes k7 (row 2) being swapped would break bank-stop order; keep k7
# after k4/k6's group and k9 last so banks stop in ascending order.
ITER2_PAIR_PHASES = [[0, 2], [1, 3, 5], [4, 6, 8], [7, 9]]
ITER2_CHUNKS = [(p, k, J_OF[k])
                for phase in ITER2_PAIR_PHASES
                for k in phase
                for p in "ABC"]

# last iter-1 row whose matmuls read activation block j (for a1 WAR)
LAST_ROW_READING = {j: min(j + 1, NUM_BLOCKS - 1) for j in range(NUM_BLOCKS)}


def _bank_schedule(chunks):
    """first/last (pass,pair,ot) touches per PSUM bank + stop order."""
    first, last = {}, {}
    for ci, (p, k, j) in enumerate(chunks):
        for ot in range(NOT):
            g = 2 * ROW_OF[k] + ot
            first.setdefault(g, (ci, ot))
            last[g] = (ci, ot)
    stop_order = sorted(last, key=lambda g: (last[g][0], last[g][1]))
    return first, last, stop_order


I1_FIRST, I1_LAST, I1_STOPS = _bank_schedule(ITER1_CHUNKS)
I2_FIRST, I2_LAST, I2_STOPS = _bank_schedule(ITER2_CHUNKS)
assert I1_STOPS == list(range(8)), I1_STOPS
# pe_sem value after bank g's stop, per iteration
I1_STOPV = {g: I1_STOPS.index(g) + 1 for g in range(8)}
I2_STOPV = {g: 8 + I2_STOPS.index(g) + 1 for g in range(8)}


def build_nc(mock_cc=False):
    nc = bass.Bass(num_devices=N_CORES)

    wh = nc.dram_tensor("wh", [10, P, NET, OSL], E4, kind="ExternalInput")
    wl = nc.dram_tensor("wl", [10, P, NET, OSL], E5, kind="ExternalInput")
    ah = nc.dram_tensor("ah", [NUM_BLOCKS, P, NET, BATCH], E4, kind="ExternalInput")
    al = nc.dram_tensor("al", [NUM_BLOCKS, P, NET, BATCH], E5, kind="ExternalInput")
    b1 = nc.dram_tensor("b1", [P, 2 * NUM_BLOCKS], F32, kind="ExternalInput")
    b2 = nc.dram_tensor("b2", [P, 2 * NUM_BLOCKS], F32, kind="ExternalInput")
    y_out = nc.dram_tensor("y", [NUM_BLOCKS, NOT, P, BATCH], BF, kind="ExternalOutput")

    cc_inh = nc.dram_tensor("cc_inh", [NUM_BLOCKS, NOT, P, BATCH], E4)
    cc_inl = nc.dram_tensor("cc_inl", [NUM_BLOCKS, NOT, P, BATCH], E5)
    cc_outh = nc.dram_tensor("cc_outh", [NUM_BLOCKS, BLOCK_SIZE, BATCH], E4,
                             addr_space="Shared")
    cc_outl = nc.dram_tensor("cc_outl", [NUM_BLOCKS, BLOCK_SIZE, BATCH], E5,
                             addr_space="Shared")

    CC_BLK = 32 if mock_cc else 2    # cc_sem per gathered block

    with (
        nc.sbuf_tensor("wh_sb", [P, 10, NET, OSL], E4) as wh_sb,
        nc.sbuf_tensor("wl_sb", [P, 10, NET, OSL], E5) as wl_sb,
        nc.sbuf_tensor("ah_sb", [P, NUM_BLOCKS, NET, BATCH], E4) as ah_sb,
        nc.sbuf_tensor("al_sb", [P, NUM_BLOCKS, NET, BATCH], E5) as al_sb,
        nc.sbuf_tensor("b1_sb", [P, 2 * NUM_BLOCKS], F32) as b1_sb,
        nc.sbuf_tensor("b2_sb", [P, 2 * NUM_BLOCKS], F32) as b2_sb,
        nc.sbuf_tensor("hi_sb", [P, 2 * NUM_BLOCKS, BATCH], E4) as hi_sb,
        nc.sbuf_tensor("lo_sb", [P, 2 * NUM_BLOCKS, BATCH], E5) as lo_sb,
        nc.sbuf_tensor("rf_sb", [P, 2, BATCH], BF) as rf_sb,
        nc.sbuf_tensor("y_sb", [P, 4, BATCH], BF) as y_sb,
        nc.psum_tensor("ps", [P, 2 * NUM_BLOCKS, BATCH], F32) as ps,
        nc.Block() as block,
    ):
        import contextlib
        _st = contextlib.ExitStack()
        wsem = [_st.enter_context(nc.semaphore(f"wsem{k}")) for k in range(10)]
        asem = [_st.enter_context(nc.semaphore(f"asem{j}")) for j in range(4)]
        bias_sem = _st.enter_context(nc.semaphore("bias_sem"))
        pe_sem = _st.enter_context(nc.semaphore("pe_sem"))
        act1_sem = _st.enter_context(nc.semaphore("act1_sem"))
        act2_sem = _st.enter_context(nc.semaphore("act2_sem"))
        dve_sem = _st.enter_context(nc.semaphore("dve_sem"))
        cin_sem = _st.enter_context(nc.semaphore("cin_sem"))
        cc_sem = _st.enter_context(nc.semaphore("cc_sem"))
        out_sem = _st.enter_context(nc.semaphore("out_sem"))
        h0_sem = _st.enter_context(nc.semaphore("h0_sem"))

        @block.sync
        def _(sp: bass.BassEngine):
            def ld_w(k, hi):
                src, dst = (wh, wh_sb) if hi else (wl, wl_sb)
                sp.dma_start(dst[:, k, :, :], src[k]).then_inc(wsem[k], 16)

            def ld_a(j, hi):
                src, dst = (ah, ah_sb) if hi else (al, al_sb)
                sp.dma_start(dst[:, j, :, :], src[j]).then_inc(asem[j], 16)

            # iteration-1 loads, ordered to keep the PE fed from its delayed
            # start onward (wsem: 16=hi 32=lo; asem: 16=hi 32=lo).  The very
            # first pair/block land as et-halves (sem inc 8 each) so the PE's
            # first half-chunk can start ~2us earlier.
            HALF = NET // 2
            for h in range(2):
                sl = slice(h * HALF, (h + 1) * HALF)
                sp.dma_start(ah_sb[:, 0, sl, :], ah[0][:, sl, :]
                             ).then_inc(h0_sem if h == 0 else asem[0], 16)
                sp.dma_start(wh_sb[:, 0, sl, :], wh[0][:, sl, :]
                             ).then_inc(h0_sem if h == 0 else wsem[0], 16)
            for n, (kind, idx) in enumerate(ITER1_LOAD_ORDER[2:]):
                if kind in ("wh", "wl"):
                    ld_w(idx, kind == "wh")
                else:
                    ld_a(idx, kind == "ah")
                if n == 0:
                    sp.dma_start(b1_sb[:, :], b1[:, :]).then_inc(bias_sem, 16)
                    sp.dma_start(b2_sb[:, :], b2[:, :]).then_inc(bias_sem, 16)

            # iteration-1 activations out to the collective, per block
            def cc_write(i):
                sp.wait_ge(dve_sem, 2 * (i + 1))
                for ot in range(NOT):
                    sp.dma_start(cc_inh[i, ot], hi_sb[:, 2 * i + ot, :]
                                 ).then_inc(cin_sem, 16)
                for ot in range(NOT):
                    sp.dma_start(cc_inl[i, ot], lo_sb[:, 2 * i + ot, :]
                                 ).then_inc(cin_sem, 16)

            # gathered re-loads (asem: 48=hi 64=lo); overwrites iter-1 slot j
            def a1_load(j):
                if not mock_cc:
                    sp.wait_ge(cc_sem, CC_BLK * (j + 1))
                sp.wait_ge(pe_sem, 2 * (LAST_ROW_READING[j] + 1))
                sp.dma_start(
                    ah_sb[:, j, :, :],
                    cc_outh[j].rearrange("(et p) b -> p et b", p=P),
                ).then_inc(asem[j], 16)
                sp.dma_start(
                    al_sb[:, j, :, :],
                    cc_outl[j].rearrange("(et p) b -> p et b", p=P),
                ).then_inc(asem[j], 16)

            for i in range(NUM_BLOCKS):
                cc_write(i)
                if mock_cc:
                    # collective stand-in on the sp queue: a local DMA of the
                    # same byte volume.  DMA-queue FIFO order makes the
                    # cc_in -> mock -> a1 data chain safe without semaphores.
                    sp.dma_start(
                        cc_outh[i, 0:NOT * P],
                        cc_inh[i].rearrange("t p b -> (t p) b"),
                    ).then_inc(cc_sem, 16)
                    sp.dma_start(
                        cc_outl[i, 0:NOT * P],
                        cc_inl[i].rearrange("t p b -> (t p) b"),
                    ).then_inc(cc_sem, 16)
                a1_load(i)

            for n, g in enumerate(I2_STOPS):
                i, ot = g // 2, g % 2
                sp.wait_ge(act2_sem, n + 1)
                sp.dma_start(y_out[i, ot], y_sb[:, n % 4, :]).then_inc(out_sem, 16)

        if not mock_cc:
            @block.gpsimd
            def _(gp: bass.BassGpSimd):
                for i in range(NUM_BLOCKS):
                    gp.wait_ge(cin_sem, 64 * (i + 1))
                    gp.collective_compute(
                        "AllGather",
                        mybir.AluOpType.bypass,
                        replica_groups=[list(range(N_CORES))],
                        ins=[cc_inh[i].opt()],
                        outs=[cc_outh[i].opt()],
                    ).then_inc(cc_sem, 1)
                    gp.collective_compute(
                        "AllGather",
                        mybir.AluOpType.bypass,
                        replica_groups=[list(range(N_CORES))],
                        ins=[cc_inl[i].opt()],
                        outs=[cc_outl[i].opt()],
                    ).then_inc(cc_sem, 1)

        @block.tensor
        def _(pe: bass.BassTensorEngine):
            DR = mybir.MatmulPerfMode.DoubleRow
            started = set()
            for it, chunks, first, last, stopv in (
                    (0, ITER1_CHUNKS, I1_FIRST, I1_LAST, I1_STOPV),
                    (1, ITER2_CHUNKS, I2_FIRST, I2_LAST, I2_STOPV)):
                for ci, (pss, k, j) in enumerate(chunks):
                    for ot in range(NOT):
                        g = 2 * ROW_OF[k] + ot
                        # operand arrival waits double as enqueue pacing
                        if it == 0:
                            if pss == "A":
                                if k == 0 and ot == 0:  # ah0/wh0 et-halved
                                    pe.wait_ge(h0_sem, 32)
                                else:
                                    pe.wait_ge(wsem[k], 16)
                                    pe.wait_ge(asem[j], 16)
                            elif pss == "B":
                                pe.wait_ge(wsem[k], 32)
                            else:
                                pe.wait_ge(asem[j], 32)
                        else:
                            pe.wait_ge(asem[j], 48 if pss in "AB" else 64)
                        if it == 1 and (it, g) not in started:
                            pe.wait_ge(act1_sem, 2 * g + 2)  # PSUM WAR vs act1
                        xs = ah_sb if pss in "AB" else al_sb
                        ws = wh_sb if pss in "AC" else wl_sb
                        is_last = last[g] == (ci, ot)
                        n_instr = NPAIR
                        if it == 1 and pss in "BC":
                            n_instr = NPAIR - DROP2 // 2
                        for ei in range(n_instr):
                            if it == 0 and pss == "A" and k == 0 \
                                    and ot == 0 and ei == NPAIR // 2:
                                pe.wait_ge(wsem[0], 16)
                                pe.wait_ge(asem[0], 16)
                            et = 2 * ei
                            mm = pe.matmul(
                                ps[:, g, :],
                                ws[:, k, et:et + 2, ot * P:(ot + 1) * P],
                                xs[:, j, et:et + 2, :],
                                start=(it, g) not in started,
                                stop=is_last and ei == n_instr - 1,
                                perf_mode=DR,
                                skip_group_check=True,
                            )
                            started.add((it, g))
                            if is_last and ei == n_instr - 1:
                                mm.then_inc(pe_sem, 1)

        @block.scalar
        def _(ac: bass.BassScalarEngine):
            Relu = mybir.ActivationFunctionType.Relu
            ac.wait_ge(bias_sem, 32)
            for g in range(2 * NUM_BLOCKS):
                ac.wait_ge(pe_sem, I1_STOPV[g])
                ac.activation(hi_sb[:, g, :], ps[:, g, :], Relu,
                              bias=b1_sb[:, g:g + 1],
                              scale=1.0 / SW).then_inc(act1_sem, 1)
                if g >= 2:
                    ac.wait_ge(dve_sem, g - 1)  # rf buf WAR vs DVE of g-2
                ac.activation(rf_sb[:, g % 2, :], ps[:, g, :], Relu,
                              bias=b1_sb[:, g:g + 1],
                              scale=1.0 / SW).then_inc(act1_sem, 1)
            for n, g in enumerate(I2_STOPS):
                ac.wait_ge(pe_sem, I2_STOPV[g])
                if n >= 4:
                    ac.wait_ge(out_sem, 16 * (n - 3))  # y buf WAR vs store
                ac.activation(y_sb[:, n % 4, :], ps[:, g, :], Relu,
                              bias=b2_sb[:, g:g + 1],
                              scale=1.0 / (SX * SW)).then_inc(act2_sem, 1)

        @block.vector
        def _(ve):
            for g in range(2 * NUM_BLOCKS):
                ve.wait_ge(act1_sem, 2 * g + 2)
                ve.tensor_sub(lo_sb[:, g, :], rf_sb[:, g % 2, :],
                              hi_sb[:, g, :]).then_inc(dve_sem, 1)

    return nc


def _prep_inputs(X, W, b):
    """Host-side shard/layout prep (pure numpy, per-core views)."""
    Xs = np.ascontiguousarray(
        (X.astype(np.float32) * SX)
        .reshape(NUM_BLOCKS, BATCH, NET, P).transpose(0, 3, 2, 1))
    ah = Xs.astype(E4np)
    al = (Xs - ah.astype(np.float32)).astype(E5np)

    Ws = W.astype(np.float32) * SW
    Whi = Ws.astype(E4np)
    Wlo = (Ws - Whi.astype(np.float32)).astype(E5np)

    B = np.zeros((NUM_BLOCKS, BLOCK_SIZE), dtype=np.float32)
    for k, (i, _) in enumerate(BLOCK_PAIRS):
        B[i] += b[k]

    in_maps = []
    for c in range(N_CORES):
        sl = slice(c * OSL, (c + 1) * OSL)
        whc = np.ascontiguousarray(
            Whi[:, sl, :].reshape(10, OSL, NET, P).transpose(0, 3, 2, 1))
        wlc = np.ascontiguousarray(
            Wlo[:, sl, :].reshape(10, OSL, NET, P).transpose(0, 3, 2, 1))
        # bias per group column g = 2*i + ot: value for partition p is
        # B[i, c*256 + ot*128 + p]
        bcols = B[:, sl].reshape(2 * NUM_BLOCKS, P).T   # [p, g]
        in_maps.append({
            "wh": whc, "wl": wlc, "ah": ah, "al": al,
            "b1": np.ascontiguousarray(bcols * SX, dtype=np.float32),
            "b2": np.ascontiguousarray(bcols, dtype=np.float32),
        })
    return in_maps


_CACHE = {}


def kernel(X, W, b):
    X = np.asarray(X, dtype=np.float32)
    W = np.asarray(W, dtype=np.float32)
    b = np.asarray(b, dtype=np.float32)
    in_maps = _prep_inputs(X, W, b)
    if "nc" not in _CACHE:
        _CACHE["nc"] = build_nc()
    res = run_bass_kernel_spmd(_CACHE["nc"], in_maps,
                               core_ids=list(range(N_CORES)))
    out = np.empty((NUM_BLOCKS, BATCH, BLOCK_SIZE), dtype=np.float32)
    for c in range(N_CORES):
        y = res.results[c]["y"].astype(np.float32)    # [4, 2, 128, 512] bf16
        out[:, :, c * OSL:(c + 1) * OSL] = y.transpose(0, 3, 1, 2).reshape(
            NUM_BLOCKS, BATCH, OSL)
    return out


# revision 29
# speedup vs baseline: 1.0075x; 1.0007x over previous
"""Block-tridiagonal iterative MLP on 8 TRN2 NeuronCores.

Tensor-parallel split of every W block along the output-feature dim (256
features per core). All matmuls run in fp8 with DoubleRow perf mode (2
contraction tiles per instruction at 0.5 cycles/row = 2x bf16 FLOP rate).
Accuracy is recovered with a residual split: every operand T is stored as
hi = e4m3(T*s) plus lo = e5m2(T*s - hi) at the SAME scale (e5m2's wider
exponent absorbs the 2^-4 magnitude drop), so the three cross terms
hi@Whi + hi@Wlo + lo@Whi accumulate in a single PSUM group with no fixup.
Iteration 2 drops the lo-correction on 2 of 16 contraction tiles (DROP2),
trading a measured rel err of 0.33% -> 1.40% (gate 2%) for ~4us.

Schedule: iteration 1 runs row-major (each block row finishes early so its
hi/lo activations can be AllGathered per block while later rows compute);
iteration 2 orders block-pairs by gathered-input availability (all pairs
contracting block 0 first, then block 1, ...), which lets the Tensor
engine run the inter-iteration boundary without a single idle cycle.
The relu, bias add, and fp8 re-quantization of iteration-1 outputs run on
the Scalar and Vector engines, fully overlapped with the Tensor engine.
"""
import sys

sys.path.insert(0, "/opt/trn_rl_repo")

import numpy as np
import ml_dtypes

import concourse.bass as bass
import concourse.mybir as mybir
from concourse.bass_utils import run_bass_kernel_spmd

N_CORES = 8
NUM_BLOCKS = 4
BLOCK_SIZE = 2048
BATCH = 512
BLOCK_PAIRS = [(0, 0), (0, 1), (1, 0), (1, 1), (1, 2),
               (2, 1), (2, 2), (2, 3), (3, 2), (3, 3)]
ROWS = {i: [(k, j) for k, (ii, j) in enumerate(BLOCK_PAIRS) if ii == i]
        for i in range(NUM_BLOCKS)}
ROW_OF = {k: i for k, (i, _) in enumerate(BLOCK_PAIRS)}
J_OF = {k: j for k, (_, j) in enumerate(BLOCK_PAIRS)}

P = 128
OSL = BLOCK_SIZE // N_CORES          # 256 out features per core
NOT = OSL // P                       # 2 output tiles (PSUM groups) per block
NET = BLOCK_SIZE // P                # 16 contraction tiles
NPAIR = NET // 2                     # 8 DoubleRow instructions per pass
SX = 16.0                            # activation scale into fp8 units
SW = 32.0                            # weight scale into fp8 units
DROP2 = 2                            # k-tiles (of 16) dropped from iter-2's
                                     # lo-correction passes; rel err 0.33%->1.40%
                                     # (measured on hw), 30% under the 2e-2 gate
E4 = mybir.dt.float8e4
E5 = mybir.dt.float8e5
BF = mybir.dt.bfloat16
F32 = mybir.dt.float32
E4np = ml_dtypes.float8_e4m3
E5np = ml_dtypes.float8_e5m2

# iteration-1 chunk schedule: row-major-ish, passes A (hi@Whi), B (hi@Wlo),
# C (lo@Whi); a chunk is (pass, pair) and emits 16 DoubleRow matmuls.  The
# order is matched to DMA arrival: C chunks of pairs contracting an
# already-resident block serve as fillers while the next loads land.
ITER1_CHUNK_ORDER = [
    ("A", 0), ("A", 2), ("A", 1), ("B", 0), ("B", 1), ("B", 2), ("A", 3),
    ("C", 0), ("B", 3), ("C", 1), ("C", 2), ("C", 3), ("A", 4), ("B", 4),
    ("A", 5), ("C", 4), ("B", 5), ("C", 5), ("A", 6), ("B", 6), ("C", 6),
    ("A", 7), ("B", 7), ("C", 7),
    ("A", 8), ("A", 9), ("B", 8), ("B", 9), ("C", 8), ("C", 9),
]
ITER1_CHUNKS = [(p, k, J_OF[k]) for p, k in ITER1_CHUNK_ORDER]
ITER1_LOAD_ORDER = [
    ("ah", 0), ("wh", 0), ("wh", 2), ("ah", 1), ("wh", 1),
    ("wl", 0), ("wl", 1), ("wl", 2), ("wh", 3), ("al", 0), ("wl", 3),
    ("al", 1), ("ah", 2), ("wh", 4), ("wl", 4), ("wh", 5), ("al", 2),
    ("wl", 5), ("wh", 6), ("wl", 6), ("ah", 3), ("wh", 7), ("wl", 7),
    ("al", 3), ("wh", 8), ("wh", 9), ("wl", 8), ("wl", 9),
]
# iteration-2: pairs grouped by which gathered block they contract, so the
# PE never waits on the AllGather chain.  Within the last group k9 (row 3)
# precedes k7 (row 2) being swapped would break bank-stop order; keep k7
# after k4/k6's group and k9 last so banks stop in ascending order.
ITER2_PAIR_PHASES = [[0, 2], [1, 3, 5], [4, 6, 8], [7, 9]]
ITER2_CHUNKS = [(p, k, J_OF[k])
                for phase in ITER2_PAIR_PHASES
                for k in phase
                for p in "ABC"]

# explicit DMA issue sequence: (kind, idx) whole loads or (kind, idx, half)
# et-halves; first halves of paired operands are interleaved so the PE's
# first instructions start as early as possible.
ITER1_LOAD_SEQ = [
    ("ah", 0, 0), ("wh", 0, 0), ("ah", 0, 1), ("wh", 0, 1),
    ("wh", 2),
    ("ah", 1, 0), ("wh", 1, 0), ("ah", 1, 1), ("wh", 1, 1),
    ("wl", 0), ("wl", 1), ("wl", 2), ("wh", 3), ("al", 0), ("wl", 3),
    ("al", 1, 0), ("al", 1, 1),
    ("ah", 2), ("wh", 4), ("wl", 4), ("wh", 5), ("al", 2),
    ("wl", 5), ("wh", 6), ("wl", 6), ("ah", 3), ("wh", 7), ("wl", 7),
    ("al", 3), ("wh", 8), ("wh", 9), ("wl", 8), ("wl", 9),
]

# head loads split into et-halves (first half -> h0_sem); H0_THR[chunk-key]
# gives the cumulative h0_sem value guaranteeing that chunk's first half.
HALF_LOADS = [("ah", 0), ("wh", 0), ("ah", 1), ("wh", 1), ("al", 1)]
_h0 = {}
_n = 0
for _ld in [("ah", 0), ("wh", 0), ("ah", 1), ("wh", 1), ("al", 1)]:
    _n += 1
    _h0[_ld] = 16 * _n
H0_THR = {("A", 0): max(_h0[("ah", 0)], _h0[("wh", 0)]),
          ("A", 1): max(_h0[("ah", 1)], _h0[("wh", 1)]),
          ("C", 1): _h0[("al", 1)]}

# last iter-1 row whose matmuls read activation block j (for a1 WAR)
LAST_ROW_READING = {j: min(j + 1, NUM_BLOCKS - 1) for j in range(NUM_BLOCKS)}


def _bank_schedule(chunks):
    """first/last (pass,pair,ot) touches per PSUM bank + stop order."""
    first, last = {}, {}
    for ci, (p, k, j) in enumerate(chunks):
        for ot in range(NOT):
            g = 2 * ROW_OF[k] + ot
            first.setdefault(g, (ci, ot))
            last[g] = (ci, ot)
    stop_order = sorted(last, key=lambda g: (last[g][0], last[g][1]))
    return first, last, stop_order


I1_FIRST, I1_LAST, I1_STOPS = _bank_schedule(ITER1_CHUNKS)
I2_FIRST, I2_LAST, I2_STOPS = _bank_schedule(ITER2_CHUNKS)
assert I1_STOPS == list(range(8)), I1_STOPS
# pe_sem value after bank g's stop, per iteration
I1_STOPV = {g: I1_STOPS.index(g) + 1 for g in range(8)}
I2_STOPV = {g: 8 + I2_STOPS.index(g) + 1 for g in range(8)}


def build_nc(mock_cc=False):
    nc = bass.Bass(num_devices=N_CORES)

    wh = nc.dram_tensor("wh", [10, P, NET, OSL], E4, kind="ExternalInput")
    wl = nc.dram_tensor("wl", [10, P, NET, OSL], E5, kind="ExternalInput")
    ah = nc.dram_tensor("ah", [NUM_BLOCKS, P, NET, BATCH], E4, kind="ExternalInput")
    al = nc.dram_tensor("al", [NUM_BLOCKS, P, NET, BATCH], E5, kind="ExternalInput")
    b1 = nc.dram_tensor("b1", [P, 2 * NUM_BLOCKS], F32, kind="ExternalInput")
    b2 = nc.dram_tensor("b2", [P, 2 * NUM_BLOCKS], F32, kind="ExternalInput")
    y_out = nc.dram_tensor("y", [NUM_BLOCKS, NOT, P, BATCH], BF, kind="ExternalOutput")

    cc_inh = nc.dram_tensor("cc_inh", [NUM_BLOCKS, NOT, P, BATCH], E4)
    cc_inl = nc.dram_tensor("cc_inl", [NUM_BLOCKS, NOT, P, BATCH], E5)
    cc_outh = nc.dram_tensor("cc_outh", [NUM_BLOCKS, BLOCK_SIZE, BATCH], E4,
                             addr_space="Shared")
    cc_outl = nc.dram_tensor("cc_outl", [NUM_BLOCKS, BLOCK_SIZE, BATCH], E5,
                             addr_space="Shared")

    CC_BLK = 32 if mock_cc else 2    # cc_sem per gathered block

    with (
        nc.sbuf_tensor("wh_sb", [P, 10, NET, OSL], E4) as wh_sb,
        nc.sbuf_tensor("wl_sb", [P, 10, NET, OSL], E5) as wl_sb,
        nc.sbuf_tensor("ah_sb", [P, NUM_BLOCKS, NET, BATCH], E4) as ah_sb,
        nc.sbuf_tensor("al_sb", [P, NUM_BLOCKS, NET, BATCH], E5) as al_sb,
        nc.sbuf_tensor("b1_sb", [P, 2 * NUM_BLOCKS], F32) as b1_sb,
        nc.sbuf_tensor("b2_sb", [P, 2 * NUM_BLOCKS], F32) as b2_sb,
        nc.sbuf_tensor("hi_sb", [P, 2 * NUM_BLOCKS, BATCH], E4) as hi_sb,
        nc.sbuf_tensor("lo_sb", [P, 2 * NUM_BLOCKS, BATCH], E5) as lo_sb,
        nc.sbuf_tensor("rf_sb", [P, 2, BATCH], BF) as rf_sb,
        nc.sbuf_tensor("y_sb", [P, 4, BATCH], BF) as y_sb,
        nc.psum_tensor("ps", [P, 2 * NUM_BLOCKS, BATCH], F32) as ps,
        nc.Block() as block,
    ):
        import contextlib
        _st = contextlib.ExitStack()
        wsem = [_st.enter_context(nc.semaphore(f"wsem{k}")) for k in range(10)]
        asem = [_st.enter_context(nc.semaphore(f"asem{j}")) for j in range(4)]
        bias_sem = _st.enter_context(nc.semaphore("bias_sem"))
        pe_sem = _st.enter_context(nc.semaphore("pe_sem"))
        act1_sem = _st.enter_context(nc.semaphore("act1_sem"))
        act2_sem = _st.enter_context(nc.semaphore("act2_sem"))
        dve_sem = _st.enter_context(nc.semaphore("dve_sem"))
        cin_sem = _st.enter_context(nc.semaphore("cin_sem"))
        cc_sem = _st.enter_context(nc.semaphore("cc_sem"))
        out_sem = _st.enter_context(nc.semaphore("out_sem"))
        h0_sem = _st.enter_context(nc.semaphore("h0_sem"))

        @block.sync
        def _(sp: bass.BassEngine):
            def ld_w(k, hi):
                src, dst = (wh, wh_sb) if hi else (wl, wl_sb)
                sp.dma_start(dst[:, k, :, :], src[k]).then_inc(wsem[k], 16)

            def ld_a(j, hi):
                src, dst = (ah, ah_sb) if hi else (al, al_sb)
                sp.dma_start(dst[:, j, :, :], src[j]).then_inc(asem[j], 16)

            # iteration-1 loads, ordered to keep the PE fed from its delayed
            # start onward (wsem: 16=hi 32=lo; asem: 16=hi 32=lo).  The head
            # loads whose arrival paces the PE land as et-halves: the first
            # half signals h0_sem (cumulative), the second the normal sem, so
            # all downstream thresholds are unchanged.
            HALF = NET // 2

            def ld_half(kind, idx, h):
                src, dst, sem = {
                    "ah": (ah, ah_sb, asem[idx]),
                    "al": (al, al_sb, asem[idx]),
                    "wh": (wh, wh_sb, wsem[idx]),
                    "wl": (wl, wl_sb, wsem[idx]),
                }[kind]
                sl = slice(h * HALF, (h + 1) * HALF)
                sp.dma_start(dst[:, idx, sl, :], src[idx][:, sl, :]
                             ).then_inc(h0_sem if h == 0 else sem, 16)

            for n, ld in enumerate(ITER1_LOAD_SEQ):
                if len(ld) == 3:
                    ld_half(*ld)
                elif ld[0] in ("wh", "wl"):
                    ld_w(ld[1], ld[0] == "wh")
                else:
                    ld_a(ld[1], ld[0] == "ah")
                if n == 4:
                    sp.dma_start(b1_sb[:, :], b1[:, :]).then_inc(bias_sem, 16)
                    sp.dma_start(b2_sb[:, :], b2[:, :]).then_inc(bias_sem, 16)

            # iteration-1 activations out to the collective, per block
            def cc_write(i):
                sp.wait_ge(dve_sem, 2 * (i + 1))
                for ot in range(NOT):
                    sp.dma_start(cc_inh[i, ot], hi_sb[:, 2 * i + ot, :]
                                 ).then_inc(cin_sem, 16)
                for ot in range(NOT):
                    sp.dma_start(cc_inl[i, ot], lo_sb[:, 2 * i + ot, :]
                                 ).then_inc(cin_sem, 16)

            # gathered re-loads (asem: 48=hi 64=lo); overwrites iter-1 slot j
            def a1_load(j):
                if not mock_cc:
                    sp.wait_ge(cc_sem, CC_BLK * (j + 1))
                sp.wait_ge(pe_sem, 2 * (LAST_ROW_READING[j] + 1))
                sp.dma_start(
                    ah_sb[:, j, :, :],
                    cc_outh[j].rearrange("(et p) b -> p et b", p=P),
                ).then_inc(asem[j], 16)
                sp.dma_start(
                    al_sb[:, j, :, :],
                    cc_outl[j].rearrange("(et p) b -> p et b", p=P),
                ).then_inc(asem[j], 16)

            for i in range(NUM_BLOCKS):
                cc_write(i)
                if mock_cc:
                    # collective stand-in on the sp queue: a local DMA of the
                    # same byte volume.  DMA-queue FIFO order makes the
                    # cc_in -> mock -> a1 data chain safe without semaphores.
                    sp.dma_start(
                        cc_outh[i, 0:NOT * P],
                        cc_inh[i].rearrange("t p b -> (t p) b"),
                    ).then_inc(cc_sem, 16)
                    sp.dma_start(
                        cc_outl[i, 0:NOT * P],
                        cc_inl[i].rearrange("t p b -> (t p) b"),
                    ).then_inc(cc_sem, 16)
                a1_load(i)

            for n, g in enumerate(I2_STOPS):
                i, ot = g // 2, g % 2
                sp.wait_ge(act2_sem, n + 1)
                sp.dma_start(y_out[i, ot], y_sb[:, n % 4, :]).then_inc(out_sem, 16)

        if not mock_cc:
            @block.gpsimd
            def _(gp: bass.BassGpSimd):
                for i in range(NUM_BLOCKS):
                    gp.wait_ge(cin_sem, 64 * (i + 1))
                    gp.collective_compute(
                        "AllGather",
                        mybir.AluOpType.bypass,
                        replica_groups=[list(range(N_CORES))],
                        ins=[cc_inh[i].opt()],
                        outs=[cc_outh[i].opt()],
                    ).then_inc(cc_sem, 1)
                    gp.collective_compute(
                        "AllGather",
                        mybir.AluOpType.bypass,
                        replica_groups=[list(range(N_CORES))],
                        ins=[cc_inl[i].opt()],
                        outs=[cc_outl[i].opt()],
                    ).then_inc(cc_sem, 1)

        @block.tensor
        def _(pe: bass.BassTensorEngine):
            DR = mybir.MatmulPerfMode.DoubleRow
            started = set()
            for it, chunks, first, last, stopv in (
                    (0, ITER1_CHUNKS, I1_FIRST, I1_LAST, I1_STOPV),
                    (1, ITER2_CHUNKS, I2_FIRST, I2_LAST, I2_STOPV)):
                for ci, (pss, k, j) in enumerate(chunks):
                    for ot in range(NOT):
                        g = 2 * ROW_OF[k] + ot
                        # operand arrival waits double as enqueue pacing
                        halved = it == 0 and ot == 0 and (pss, k) in H0_THR
                        if it == 0:
                            if halved:
                                pe.wait_ge(h0_sem, H0_THR[(pss, k)])
                            elif pss == "A":
                                pe.wait_ge(wsem[k], 16)
                                pe.wait_ge(asem[j], 16)
                            elif pss == "B":
                                pe.wait_ge(wsem[k], 32)
                            else:
                                pe.wait_ge(asem[j], 32)
                        else:
                            pe.wait_ge(asem[j], 48 if pss in "AB" else 64)
                        if it == 1 and (it, g) not in started:
                            pe.wait_ge(act1_sem, 2 * g + 2)  # PSUM WAR vs act1
                        xs = ah_sb if pss in "AB" else al_sb
                        ws = wh_sb if pss in "AC" else wl_sb
                        is_last = last[g] == (ci, ot)
                        n_instr = NPAIR
                        if it == 1 and pss in "BC":
                            n_instr = NPAIR - DROP2 // 2
                        for ei in range(n_instr):
                            if halved and ei == NPAIR // 2:
                                if pss == "A":
                                    pe.wait_ge(wsem[k], 16)
                                    pe.wait_ge(asem[j], 16)
                                else:
                                    pe.wait_ge(asem[j], 32)
                            et = 2 * ei
                            mm = pe.matmul(
                                ps[:, g, :],
                                ws[:, k, et:et + 2, ot * P:(ot + 1) * P],
                                xs[:, j, et:et + 2, :],
                                start=(it, g) not in started,
                                stop=is_last and ei == n_instr - 1,
                                perf_mode=DR,
                                skip_group_check=True,
                            )
                            started.add((it, g))
                            if is_last and ei == n_instr - 1:
                                mm.then_inc(pe_sem, 1)

        @block.scalar
        def _(ac: bass.BassScalarEngine):
            Relu = mybir.ActivationFunctionType.Relu
            ac.wait_ge(bias_sem, 32)
            for g in range(2 * NUM_BLOCKS):
                ac.wait_ge(pe_sem, I1_STOPV[g])
                ac.activation(hi_sb[:, g, :], ps[:, g, :], Relu,
                              bias=b1_sb[:, g:g + 1],
                              scale=1.0 / SW).then_inc(act1_sem, 1)
                if g >= 2:
                    ac.wait_ge(dve_sem, g - 1)  # rf buf WAR vs DVE of g-2
                ac.activation(rf_sb[:, g % 2, :], ps[:, g, :], Relu,
                              bias=b1_sb[:, g:g + 1],
                              scale=1.0 / SW).then_inc(act1_sem, 1)
            for n, g in enumerate(I2_STOPS):
                ac.wait_ge(pe_sem, I2_STOPV[g])
                if n >= 4:
                    ac.wait_ge(out_sem, 16 * (n - 3))  # y buf WAR vs store
                ac.activation(y_sb[:, n % 4, :], ps[:, g, :], Relu,
                              bias=b2_sb[:, g:g + 1],
                              scale=1.0 / (SX * SW)).then_inc(act2_sem, 1)

        @block.vector
        def _(ve):
            for g in range(2 * NUM_BLOCKS):
                ve.wait_ge(act1_sem, 2 * g + 2)
                ve.tensor_sub(lo_sb[:, g, :], rf_sb[:, g % 2, :],
                              hi_sb[:, g, :]).then_inc(dve_sem, 1)

    return nc


def _prep_inputs(X, W, b):
    """Host-side shard/layout prep (pure numpy, per-core views)."""
    Xs = np.ascontiguousarray(
        (X.astype(np.float32) * SX)
        .reshape(NUM_BLOCKS, BATCH, NET, P).transpose(0, 3, 2, 1))
    ah = Xs.astype(E4np)
    al = (Xs - ah.astype(np.float32)).astype(E5np)

    Ws = W.astype(np.float32) * SW
    Whi = Ws.astype(E4np)
    Wlo = (Ws - Whi.astype(np.float32)).astype(E5np)

    B = np.zeros((NUM_BLOCKS, BLOCK_SIZE), dtype=np.float32)
    for k, (i, _) in enumerate(BLOCK_PAIRS):
        B[i] += b[k]

    in_maps = []
    for c in range(N_CORES):
        sl = slice(c * OSL, (c + 1) * OSL)
        whc = np.ascontiguousarray(
            Whi[:, sl, :].reshape(10, OSL, NET, P).transpose(0, 3, 2, 1))
        wlc = np.ascontiguousarray(
            Wlo[:, sl, :].reshape(10, OSL, NET, P).transpose(0, 3, 2, 1))
        # bias per group column g = 2*i + ot: value for partition p is
        # B[i, c*256 + ot*128 + p]
        bcols = B[:, sl].reshape(2 * NUM_BLOCKS, P).T   # [p, g]
        in_maps.append({
            "wh": whc, "wl": wlc, "ah": ah, "al": al,
            "b1": np.ascontiguousarray(bcols * SX, dtype=np.float32),
            "b2": np.ascontiguousarray(bcols, dtype=np.float32),
        })
    return in_maps


_CACHE = {}


def kernel(X, W, b):
    X = np.asarray(X, dtype=np.float32)
    W = np.asarray(W, dtype=np.float32)
    b = np.asarray(b, dtype=np.float32)
    in_maps = _prep_inputs(X, W, b)
    if "nc" not in _CACHE:
        _CACHE["nc"] = build_nc()
    res = run_bass_kernel_spmd(_CACHE["nc"], in_maps,
                               core_ids=list(range(N_CORES)))
    out = np.empty((NUM_BLOCKS, BATCH, BLOCK_SIZE), dtype=np.float32)
    for c in range(N_CORES):
        y = res.results[c]["y"].astype(np.float32)    # [4, 2, 128, 512] bf16
        out[:, :, c * OSL:(c + 1) * OSL] = y.transpose(0, 3, 1, 2).reshape(
            NUM_BLOCKS, BATCH, OSL)
    return out
